# revision 54
# baseline (speedup 1.0000x reference)
"""Trainium2 Bass kernel for nn_MoEFusion (multi-modal MoE fusion MLP).

Data-parallel across 8 NeuronCores: batch dim (32768) sharded into 8
slices of 4096, all weights (<1 MB) replicated. No collectives.

On-device dataflow (per core, feature-major "T" layout everywhere):
  featT [768, 4096] (host-pre-transposed, bf16) --DMA--> SBUF
  projT[m] = proj_w[m].T @ featT[m]            (PE bf16, accum K=768)
  xT = concat_m(projT + proj_b)                (ACT bias-add PSUM->SBUF, bf16)
  gateT = exp(gate_w.T @ xT + gate_b)          (PE + ACT Exp, f32r out)
  colsum via all-ones K=8 matmul (PE), reciprocal_approx_fast (DVE)
  gwT = gateT * rsum                           (DVE, f32r)
  hT[e] = relu(W1[e].T @ xT + b1[e])           (PE bf16 + ACT Relu)
  gw bcast to 128 partitions via one-hot K=8 matmul (PE, f32r)
  shT[e] = hT[e] * gw_bcast[e]                 (DVE, bf16 out)
  fusedT = exp_b2.T @ gwT (f32r) + sum_e W2[e].T @ shT[e] (bf16), one PSUM
  penT = relu(pre_w.T @ fusedT + pre_b)        (PE + ACT)
  outT = head_w.T @ penT + head_b              (PE + ACT) --DMA--> [2, 4096]
Host re-transposes/concats to [32768, 2] fp32.

Software pipeline: the l2 accumulation and pre/head of stripe s-1 are
emitted during stripe s so the PE stream never waits on the
gate-softmax chain. bf16 moving+stationary operands stream the PE at
1 col/cycle (f32r measured ~2 cyc/row on HW); the softmax/gating path
stays f32r end-to-end so gate weights keep ~fp32 precision.
"""

import sys

if "/opt/trn_rl_repo" not in sys.path:
    sys.path.insert(0, "/opt/trn_rl_repo")

from contextlib import ExitStack

import ml_dtypes
import numpy as np

# ---- problem constants (hardcoded per contract) ----
B = 32768
NCORES = 8
BL = B // NCORES  # 4096 per core
STRIPE = 512
NM = 3
NE = 8
D_IN = 768
KIN = D_IN // 128  # 6
D_P = 128
D_X = 384
KX = D_X // 128  # 3

BF16 = ml_dtypes.bfloat16

# ---- bf16 packed weight layout (columns of [128, WBFCOLS]) ----
OFF_PROJ = 0                           # [p, m*768 + k*128 + o] = proj_w[m, k*128+p, o]
OFF_W1 = OFF_PROJ + NM * KIN * 128     # 2304
OFF_W2 = OFF_W1 + NE * KX * 128        # 5376
OFF_GATE = OFF_W2 + NE * 128           # 6400
OFF_PRE = OFF_GATE + KX * NE           # 6424
OFF_HEAD = OFF_PRE + 64                # 6488
OFF_B2B = OFF_HEAD + 2                 # 6490: [p<8, o] = exp_b2[p, o]
OFF_OHB = OFF_B2B + 128                # 6618: [p<8, e*128+q] = (p==e)
WBFCOLS = OFF_OHB + NE * 128           # 7642

# ---- f32r packed weights (softmax sum, columns of [128, WRCOLS]) ----
OFF_ONES = 0                           # [p<8, 0:8] = 1.0
WRCOLS = OFF_ONES + NE                 # 8

# ---- f32 biases (columns of [128, WBCOLS]) ----
OFF_PROJB = 0
OFF_B1 = OFF_PROJB + NM
OFF_GATEB = OFF_B1 + NE
OFF_PREB = OFF_GATEB + 1
OFF_HEADB = OFF_PREB + 1
WBCOLS = OFF_HEADB + 1                 # 14


def pack_weights(inp):
    wb16 = np.zeros((128, WBFCOLS), np.float32)
    pw = np.asarray(inp["proj_w"], np.float32)
    wb16[:, OFF_PROJ:OFF_W1] = (
        pw.reshape(NM, KIN, 128, 128).transpose(2, 0, 1, 3).reshape(128, -1)
    )
    w1 = np.asarray(inp["exp_w1"], np.float32)
    wb16[:, OFF_W1:OFF_W2] = (
        w1.reshape(NE, KX, 128, 128).transpose(2, 0, 1, 3).reshape(128, -1)
    )
    w2 = np.asarray(inp["exp_w2"], np.float32)
    wb16[:, OFF_W2:OFF_GATE] = w2.transpose(1, 0, 2).reshape(128, -1)
    gw = np.asarray(inp["gate_w"], np.float32)
    wb16[:, OFF_GATE:OFF_PRE] = (
        gw.reshape(KX, 128, NE).transpose(1, 0, 2).reshape(128, -1)
    )
    wb16[:, OFF_PRE:OFF_HEAD] = np.asarray(inp["pre_w"], np.float32)
    wb16[:64, OFF_HEAD:OFF_B2B] = np.asarray(inp["head_w"], np.float32)
    wb16[:8, OFF_B2B:OFF_OHB] = np.asarray(inp["exp_b2"], np.float32)
    oh = np.zeros((8, NE, 128), np.float32)
    for e in range(NE):
        oh[e, e, :] = 1.0
    wb16[:8, OFF_OHB:WBFCOLS] = oh.reshape(8, -1)
    wb16 = wb16.astype(BF16)

    wr = np.zeros((128, WRCOLS), np.float32)
    wr[:8, OFF_ONES:OFF_ONES + NE] = 1.0

    wbias = np.zeros((128, WBCOLS), np.float32)
    wbias[:, OFF_PROJB:OFF_B1] = np.asarray(inp["proj_b"], np.float32).T
    wbias[:, OFF_B1:OFF_GATEB] = np.asarray(inp["exp_b1"], np.float32).T
    wbias[:8, OFF_GATEB] = np.asarray(inp["gate_b"], np.float32)
    wbias[:64, OFF_PREB] = np.asarray(inp["pre_b"], np.float32)
    wbias[:2, OFF_HEADB] = np.asarray(inp["head_b"], np.float32)
    return wb16, wr, wbias


def build_program(n_stripes=BL // STRIPE):
    """Build the per-core Bass program (identical on all cores)."""
    import concourse.bacc as bacc
    import concourse.mybir as mybir
    import concourse.tile as tile

    f32 = mybir.dt.float32
    f32r = mybir.dt.float32r
    bf16 = mybir.dt.bfloat16
    AF = mybir.ActivationFunctionType
    bl = n_stripes * STRIPE

    nc = bacc.Bacc(
        "TRN2",
        target_bir_lowering=False,
        debug=False,
        enable_asserts=False,
    )

    featT = nc.dram_tensor("featT", [NM, D_IN, bl], bf16, kind="ExternalInput").ap()
    wmatb = nc.dram_tensor("wmatb", [128, WBFCOLS], bf16, kind="ExternalInput").ap()
    wmatr = nc.dram_tensor("wmatr", [128, WRCOLS], f32r, kind="ExternalInput").ap()
    wbias = nc.dram_tensor("wbias", [128, WBCOLS], f32, kind="ExternalInput").ap()
    outT = nc.dram_tensor("outT", [2, bl], f32, kind="ExternalOutput").ap()

    with tile.TileContext(nc) as tc, ExitStack() as ctx:
        wp_pool = ctx.enter_context(tc.tile_pool(name="wp", bufs=1))
        feat_pool = ctx.enter_context(tc.tile_pool(name="feat", bufs=18))
        x_pool = ctx.enter_context(tc.tile_pool(name="x", bufs=6))
        gw_pool = ctx.enter_context(tc.tile_pool(name="gw", bufs=3))
        grow_pool = ctx.enter_context(tc.tile_pool(name="grow", bufs=3))
        gb_pool = ctx.enter_context(tc.tile_pool(name="gb", bufs=6))
        h_pool = ctx.enter_context(tc.tile_pool(name="h", bufs=10))
        sh_pool = ctx.enter_context(tc.tile_pool(name="sh", bufs=18))
        f_pool = ctx.enter_context(tc.tile_pool(name="f", bufs=2))
        pen_pool = ctx.enter_context(tc.tile_pool(name="pen", bufs=2))
        o_pool = ctx.enter_context(tc.tile_pool(name="o", bufs=2))

        px_pool = ctx.enter_context(tc.tile_pool(name="px", bufs=2, space="PSUM"))
        ph_pool = ctx.enter_context(tc.tile_pool(name="ph", bufs=3, space="PSUM"))
        pf_pool = ctx.enter_context(tc.tile_pool(name="pf", bufs=2, space="PSUM"))
        ps_pool = ctx.enter_context(tc.tile_pool(name="ps", bufs=1, space="PSUM"))

        # preload packed weights once (proj block first so MMs start early)
        Wb = wp_pool.tile([128, WBFCOLS], bf16)
        nc.scalar.dma_start(Wb[:, :OFF_W1], wmatb[:, :OFF_W1])
        nc.scalar.dma_start(Wb[:, OFF_W1:], wmatb[:, OFF_W1:])
        Wr = wp_pool.tile([128, WRCOLS], f32r)
        nc.scalar.dma_start(Wr[:], wmatr[:])
        Bz = wp_pool.tile([128, WBCOLS], f32)
        nc.sync.dma_start(Bz[:], wbias[:])


        def wb(off, n, parts=128):
            return Wb[:parts, off : off + n]

        def wr(off, n, parts=128):
            return Wr[:parts, off : off + n]

        def bslice(off, parts=128):
            return Bz[:parts, off : off + 1]

        featT_t = featT.rearrange("m (k p) b -> m p k b", p=128)

        pends = []  # (sh, gwT, bsl) of the previous two stripes

        def emit_l2(pend):
            sh, gwT, bsl = pend
            pf = pf_pool.tile([128, STRIPE], f32, tag="pf")
            nc.tensor.matmul(
                pf[:], wb(OFF_B2B, 128, parts=8), gwT[:],
                start=True, stop=False,
            )
            for e in range(NE):
                nc.tensor.matmul(
                    pf[:],
                    wb(OFF_W2 + e * 128, 128),
                    sh[e][:],
                    start=False,
                    stop=(e == NE - 1),
                )
            fT = f_pool.tile([128, STRIPE], bf16, tag="f")
            nc.scalar.copy(fT[:], pf[:])
            return fT

        def emit_head(fT, bsl):
            pp = ps_pool.tile([64, STRIPE], f32, tag="ps")
            nc.tensor.matmul(pp[:], wb(OFF_PRE, 64), fT[:],
                             start=True, stop=True)
            pen = pen_pool.tile([64, STRIPE], bf16, tag="pen")
            nc.scalar.activation(
                pen[:], pp[:], AF.Relu, bias=bslice(OFF_PREB, parts=64), scale=1.0
            )
            po = ps_pool.tile([2, STRIPE], f32, tag="ps")
            nc.tensor.matmul(po[:], wb(OFF_HEAD, 2, parts=64), pen[:],
                             start=True, stop=True)
            ot = o_pool.tile([2, STRIPE], f32, tag="o")
            nc.scalar.activation(
                ot[:], po[:], AF.Identity, bias=bslice(OFF_HEADB, parts=2),
                scale=1.0,
            )
            nc.scalar.dma_start(outT[:, bsl], ot[:])

        for s in range(n_stripes):
            bsl = slice(s * STRIPE, (s + 1) * STRIPE)

            # ---- load features (0.75 MB per modality) ----
            ft = []
            h2 = KIN // 2
            for m in range(NM):
                if s == 0:
                    ta = feat_pool.tile([128, h2, STRIPE], bf16, tag="feat")
                    tb = feat_pool.tile([128, KIN - h2, STRIPE], bf16, tag="feat")
                    for k in range(KIN):
                        dst = ta if k < h2 else tb
                        nc.sync.dma_start(dst[:, k % h2, :],
                                          featT_t[m, :, k, bsl])
                else:
                    ta = feat_pool.tile([128, h2, STRIPE], bf16, tag="feat")
                    nc.sync.dma_start(ta[:], featT_t[m, :, :h2, bsl])
                    tb = feat_pool.tile([128, KIN - h2, STRIPE], bf16, tag="feat")
                    nc.sync.dma_start(tb[:], featT_t[m, :, h2:, bsl])
                ft.append((ta, tb))

            # ---- per-modality projection -> xT chunks ----
            xT = []
            for m in range(NM):
                px = px_pool.tile([128, STRIPE], f32, tag="px")
                for k in range(KIN):
                    src_t = ft[m][0] if k < h2 else ft[m][1]
                    nc.tensor.matmul(
                        px[:],
                        wb(OFF_PROJ + m * KIN * 128 + k * 128, 128),
                        src_t[:, k % h2, :],
                        start=(k == 0),
                        stop=(k == KIN - 1),
                    )
                xt = x_pool.tile([128, STRIPE], bf16, tag="x")
                nc.scalar.activation(
                    xt[:], px[:], AF.Identity,
                    bias=bslice(OFF_PROJB + m), scale=1.0,
                )
                xT.append(xt)

            # ---- stage-2, two stripes back: l2 accumulation ----
            fT_prev = None
            if len(pends) == 2:
                p0 = pends.pop(0)
                fT_prev = emit_l2(p0)
                pend_bsl = p0[2]

            # ---- gate: softmax over 8 experts (f32r path) ----
            pg = ps_pool.tile([8, STRIPE], f32, tag="ps")
            for k in range(KX):
                nc.tensor.matmul(
                    pg[:],
                    wb(OFF_GATE + k * NE, NE),
                    xT[k][:],
                    start=(k == 0),
                    stop=(k == KX - 1),
                )
            eT = gw_pool.tile([8, STRIPE], f32r, tag="eT")
            nc.scalar.activation(
                eT[:], pg[:], AF.Exp, bias=bslice(OFF_GATEB, parts=8), scale=1.0
            )
            psum_s = ps_pool.tile([8, STRIPE], f32, tag="ps")
            nc.tensor.matmul(psum_s[:], wr(OFF_ONES, NE, parts=8), eT[:],
                             start=True, stop=True)
            rT = gw_pool.tile([8, STRIPE], f32, tag="rT")
            nc.vector.reciprocal_approx_fast(rT[:], psum_s[:])
            gwT = gw_pool.tile([8, STRIPE], bf16, tag="gwT")
            nc.vector.tensor_mul(gwT[:], eT[:], rT[:])

            # gather gate rows onto partition 0; broadcast on idle GPSIMD
            grow = grow_pool.tile([1, NE, STRIPE], bf16, tag="grow")
            nc.scalar.dma_start(grow[:], gwT[:])

            # ---- experts: h = relu(W1.T x + b1); sh = h * gw[e] ----
            sh = []
            for e in range(NE):
                ph = ph_pool.tile([128, STRIPE], f32, tag="ph")
                for k in range(KX):
                    nc.tensor.matmul(
                        ph[:],
                        wb(OFF_W1 + e * KX * 128 + k * 128, 128),
                        xT[k][:],
                        start=(k == 0),
                        stop=(k == KX - 1),
                    )
                h = h_pool.tile([128, STRIPE], bf16, tag="h")
                nc.scalar.activation(
                    h[:], ph[:], AF.Relu, bias=bslice(OFF_B1 + e), scale=1.0
                )
                gb = gb_pool.tile([128, STRIPE], bf16, tag="gb")
                nc.gpsimd.partition_broadcast(gb[:], grow[0:1, e, :], channels=128)
                sht = sh_pool.tile([128, STRIPE], bf16, tag="sh")
                nc.vector.tensor_mul(sht[:], h[:], gb[:])
                sh.append(sht)

            if fT_prev is not None:
                emit_head(fT_prev, pend_bsl)
            pends.append((sh, gwT, bsl))

        flush = [(emit_l2(p0), p0[2]) for p0 in pends]
        for fT, bsl_ in flush:
            emit_head(fT, bsl_)

    nc.compile()
    return nc


_PROGRAM = None


def _get_program():
    global _PROGRAM
    if _PROGRAM is None:
        _PROGRAM = build_program()
    return _PROGRAM


def make_in_maps(inputs):
    """Host-side shard + layout prep: list of 8 per-core input maps."""
    wb16, wr, wbias = pack_weights(inputs)
    feats = [
        np.asarray(inputs["feat_text"], np.float32),
        np.asarray(inputs["feat_audio"], np.float32),
        np.asarray(inputs["feat_video"], np.float32),
    ]
    in_maps = []
    for c in range(NCORES):
        sl = slice(c * BL, (c + 1) * BL)
        featT = np.stack([np.ascontiguousarray(f[sl].T) for f in feats])
        in_maps.append({
            "featT": featT.astype(BF16),
            "wmatb": wb16,
            "wmatr": wr,
            "wbias": wbias,
        })
    return in_maps


def run_on_hw(inputs, trace=False):
    from concourse.bass_utils import run_bass_kernel_spmd

    nc = _get_program()
    in_maps = make_in_maps(inputs)
    res = run_bass_kernel_spmd(
        nc, in_maps, core_ids=list(range(NCORES)), trace=trace
    )
    out = np.concatenate([r["outT"].T for r in res.results], axis=0)
    return out, res


def kernel(**inputs):
    out, _ = run_on_hw(inputs, trace=False)
    return out


# revision 55
# speedup vs baseline: 1.0413x; 1.0413x over previous
"""Trainium2 Bass kernel for nn_MoEFusion (multi-modal MoE fusion MLP).

Data-parallel across 8 NeuronCores: batch dim (32768) sharded into 8
slices of 4096, all weights (<1 MB) replicated. No collectives.

On-device dataflow (per core, feature-major "T" layout everywhere):
  featT [768, 4096] (host-pre-transposed, bf16) --DMA--> SBUF
  projT[m] = proj_w[m].T @ featT[m]            (PE bf16, accum K=768)
  xT = concat_m(projT + proj_b)                (ACT bias-add PSUM->SBUF, bf16)
  gateT = exp(gate_w.T @ xT + gate_b)          (PE + ACT Exp, f32r out)
  colsum via all-ones K=8 matmul (PE), reciprocal_approx_fast (DVE)
  gwT = gateT * rsum                           (DVE, f32r)
  hT[e] = relu(W1[e].T @ xT + b1[e])           (PE bf16 + ACT Relu)
  gw bcast to 128 partitions via one-hot K=8 matmul (PE, f32r)
  shT[e] = hT[e] * gw_bcast[e]                 (DVE, bf16 out)
  fusedT = exp_b2.T @ gwT (f32r) + sum_e W2[e].T @ shT[e] (bf16), one PSUM
  penT = relu(pre_w.T @ fusedT + pre_b)        (PE + ACT)
  outT = head_w.T @ penT + head_b              (PE + ACT) --DMA--> [2, 4096]
Host re-transposes/concats to [32768, 2] fp32.

Software pipeline: the l2 accumulation and pre/head of stripe s-1 are
emitted during stripe s so the PE stream never waits on the
gate-softmax chain. bf16 moving+stationary operands stream the PE at
1 col/cycle (f32r measured ~2 cyc/row on HW); the softmax/gating path
stays f32r end-to-end so gate weights keep ~fp32 precision.
"""

import sys

if "/opt/trn_rl_repo" not in sys.path:
    sys.path.insert(0, "/opt/trn_rl_repo")

from contextlib import ExitStack

import ml_dtypes
import numpy as np

# ---- problem constants (hardcoded per contract) ----
B = 32768
NCORES = 8
BL = B // NCORES  # 4096 per core
STRIPE = 512
NM = 3
NE = 8
D_IN = 768
KIN = D_IN // 128  # 6
D_P = 128
D_X = 384
KX = D_X // 128  # 3

BF16 = ml_dtypes.bfloat16

# ---- bf16 packed weight layout (columns of [128, WBFCOLS]) ----
OFF_PROJ = 0                           # [p, m*768 + k*128 + o] = proj_w[m, k*128+p, o]
OFF_W1 = OFF_PROJ + NM * KIN * 128     # 2304
OFF_W2 = OFF_W1 + NE * KX * 128        # 5376
OFF_GATE = OFF_W2 + NE * 128           # 6400
OFF_PRE = OFF_GATE + KX * NE           # 6424
OFF_HEAD = OFF_PRE + 64                # 6488
OFF_B2B = OFF_HEAD + 2                 # 6490: [p<8, o] = exp_b2[p, o]
OFF_OHB = OFF_B2B + 128                # 6618: [p<8, e*128+q] = (p==e)
WBFCOLS = OFF_OHB + NE * 128           # 7642

# ---- f32r packed weights (softmax sum, columns of [128, WRCOLS]) ----
OFF_ONES = 0                           # [p<8, 0:8] = 1.0
WRCOLS = OFF_ONES + NE                 # 8

# ---- f32 biases (columns of [128, WBCOLS]) ----
OFF_PROJB = 0
OFF_B1 = OFF_PROJB + NM
OFF_GATEB = OFF_B1 + NE
OFF_PREB = OFF_GATEB + 1
OFF_HEADB = OFF_PREB + 1
WBCOLS = OFF_HEADB + 1                 # 14


def pack_weights(inp):
    wb16 = np.zeros((128, WBFCOLS), np.float32)
    pw = np.asarray(inp["proj_w"], np.float32)
    wb16[:, OFF_PROJ:OFF_W1] = (
        pw.reshape(NM, KIN, 128, 128).transpose(2, 0, 1, 3).reshape(128, -1)
    )
    w1 = np.asarray(inp["exp_w1"], np.float32)
    wb16[:, OFF_W1:OFF_W2] = (
        w1.reshape(NE, KX, 128, 128).transpose(2, 0, 1, 3).reshape(128, -1)
    )
    w2 = np.asarray(inp["exp_w2"], np.float32)
    wb16[:, OFF_W2:OFF_GATE] = w2.transpose(1, 0, 2).reshape(128, -1)
    gw = np.asarray(inp["gate_w"], np.float32)
    wb16[:, OFF_GATE:OFF_PRE] = (
        gw.reshape(KX, 128, NE).transpose(1, 0, 2).reshape(128, -1)
    )
    wb16[:, OFF_PRE:OFF_HEAD] = np.asarray(inp["pre_w"], np.float32)
    wb16[:64, OFF_HEAD:OFF_B2B] = np.asarray(inp["head_w"], np.float32)
    wb16[:8, OFF_B2B:OFF_OHB] = np.asarray(inp["exp_b2"], np.float32)
    oh = np.zeros((8, NE, 128), np.float32)
    for e in range(NE):
        oh[e, e, :] = 1.0
    wb16[:8, OFF_OHB:WBFCOLS] = oh.reshape(8, -1)
    wb16 = wb16.astype(BF16)

    wr = np.zeros((128, WRCOLS), np.float32)
    wr[:8, OFF_ONES:OFF_ONES + NE] = 1.0

    wbias = np.zeros((128, WBCOLS), np.float32)
    wbias[:, OFF_PROJB:OFF_B1] = np.asarray(inp["proj_b"], np.float32).T
    wbias[:, OFF_B1:OFF_GATEB] = np.asarray(inp["exp_b1"], np.float32).T
    wbias[:8, OFF_GATEB] = np.asarray(inp["gate_b"], np.float32)
    wbias[:64, OFF_PREB] = np.asarray(inp["pre_b"], np.float32)
    wbias[:2, OFF_HEADB] = np.asarray(inp["head_b"], np.float32)
    return wb16, wr, wbias


def build_program(n_stripes=BL // STRIPE):
    """Build the per-core Bass program (identical on all cores)."""
    import concourse.bacc as bacc
    import concourse.mybir as mybir
    import concourse.tile as tile

    f32 = mybir.dt.float32
    f32r = mybir.dt.float32r
    bf16 = mybir.dt.bfloat16
    AF = mybir.ActivationFunctionType
    bl = n_stripes * STRIPE

    nc = bacc.Bacc(
        "TRN2",
        target_bir_lowering=False,
        debug=False,
        enable_asserts=False,
    )

    featT = nc.dram_tensor("featT", [NM, D_IN, bl], bf16, kind="ExternalInput").ap()
    wmatb = nc.dram_tensor("wmatb", [128, WBFCOLS], bf16, kind="ExternalInput").ap()
    wmatr = nc.dram_tensor("wmatr", [128, WRCOLS], f32r, kind="ExternalInput").ap()
    wbias = nc.dram_tensor("wbias", [128, WBCOLS], f32, kind="ExternalInput").ap()
    outT = nc.dram_tensor("outT", [2, bl], f32, kind="ExternalOutput").ap()

    with tile.TileContext(nc) as tc, ExitStack() as ctx:
        wp_pool = ctx.enter_context(tc.tile_pool(name="wp", bufs=1))
        feat_pool = ctx.enter_context(tc.tile_pool(name="feat", bufs=18))
        x_pool = ctx.enter_context(tc.tile_pool(name="x", bufs=6))
        gw_pool = ctx.enter_context(tc.tile_pool(name="gw", bufs=3))
        grow_pool = ctx.enter_context(tc.tile_pool(name="grow", bufs=3))
        gb_pool = ctx.enter_context(tc.tile_pool(name="gb", bufs=6))
        h_pool = ctx.enter_context(tc.tile_pool(name="h", bufs=10))
        sh_pool = ctx.enter_context(tc.tile_pool(name="sh", bufs=18))
        f_pool = ctx.enter_context(tc.tile_pool(name="f", bufs=2))
        pen_pool = ctx.enter_context(tc.tile_pool(name="pen", bufs=2))
        o_pool = ctx.enter_context(tc.tile_pool(name="o", bufs=2))

        px_pool = ctx.enter_context(tc.tile_pool(name="px", bufs=2, space="PSUM"))
        ph_pool = ctx.enter_context(tc.tile_pool(name="ph", bufs=3, space="PSUM"))
        pf_pool = ctx.enter_context(tc.tile_pool(name="pf", bufs=2, space="PSUM"))
        ps_pool = ctx.enter_context(tc.tile_pool(name="ps", bufs=1, space="PSUM"))

        # preload packed weights once (proj block first so MMs start early)
        Wb = wp_pool.tile([128, WBFCOLS], bf16)
        nc.scalar.dma_start(Wb[:, :OFF_W1], wmatb[:, :OFF_W1])
        nc.scalar.dma_start(Wb[:, OFF_W1:], wmatb[:, OFF_W1:])
        Wr = wp_pool.tile([128, WRCOLS], f32r)
        nc.scalar.dma_start(Wr[:], wmatr[:])
        Bz = wp_pool.tile([128, WBCOLS], f32)
        nc.scalar.dma_start(Bz[:], wbias[:])


        def wb(off, n, parts=128):
            return Wb[:parts, off : off + n]

        def wr(off, n, parts=128):
            return Wr[:parts, off : off + n]

        def bslice(off, parts=128):
            return Bz[:parts, off : off + 1]

        featT_t = featT.rearrange("m (k p) b -> m p k b", p=128)

        pends = []  # (sh, gwT, bsl) of the previous two stripes

        def emit_l2(pend):
            sh, gwT, bsl = pend
            pf = pf_pool.tile([128, STRIPE], f32, tag="pf")
            nc.tensor.matmul(
                pf[:], wb(OFF_B2B, 128, parts=8), gwT[:],
                start=True, stop=False,
            )
            for e in range(NE):
                nc.tensor.matmul(
                    pf[:],
                    wb(OFF_W2 + e * 128, 128),
                    sh[e][:],
                    start=False,
                    stop=(e == NE - 1),
                )
            fT = f_pool.tile([128, STRIPE], bf16, tag="f")
            nc.scalar.copy(fT[:], pf[:])
            return fT

        def emit_head(fT, bsl):
            pp = ps_pool.tile([64, STRIPE], f32, tag="ps")
            nc.tensor.matmul(pp[:], wb(OFF_PRE, 64), fT[:],
                             start=True, stop=True)
            pen = pen_pool.tile([64, STRIPE], bf16, tag="pen")
            nc.scalar.activation(
                pen[:], pp[:], AF.Relu, bias=bslice(OFF_PREB, parts=64), scale=1.0
            )
            po = ps_pool.tile([2, STRIPE], f32, tag="ps")
            nc.tensor.matmul(po[:], wb(OFF_HEAD, 2, parts=64), pen[:],
                             start=True, stop=True)
            ot = o_pool.tile([2, STRIPE], f32, tag="o")
            nc.scalar.activation(
                ot[:], po[:], AF.Identity, bias=bslice(OFF_HEADB, parts=2),
                scale=1.0,
            )
            nc.scalar.dma_start(outT[:, bsl], ot[:])

        for s in range(n_stripes):
            bsl = slice(s * STRIPE, (s + 1) * STRIPE)

            # ---- load features (0.75 MB per modality) ----
            ft = []
            h2 = KIN // 2
            for m in range(NM):
                ta = feat_pool.tile([128, h2, STRIPE], bf16, tag="feat")
                nc.sync.dma_start(ta[:], featT_t[m, :, :h2, bsl])
                tb = feat_pool.tile([128, KIN - h2, STRIPE], bf16, tag="feat")
                nc.sync.dma_start(tb[:], featT_t[m, :, h2:, bsl])
                ft.append((ta, tb))

            # ---- per-modality projection -> xT chunks ----
            xT = []
            for m in range(NM):
                px = px_pool.tile([128, STRIPE], f32, tag="px")
                for k in range(KIN):
                    src_t = ft[m][0] if k < h2 else ft[m][1]
                    nc.tensor.matmul(
                        px[:],
                        wb(OFF_PROJ + m * KIN * 128 + k * 128, 128),
                        src_t[:, k % h2, :],
                        start=(k == 0),
                        stop=(k == KIN - 1),
                    )
                xt = x_pool.tile([128, STRIPE], bf16, tag="x")
                nc.scalar.activation(
                    xt[:], px[:], AF.Identity,
                    bias=bslice(OFF_PROJB + m), scale=1.0,
                )
                xT.append(xt)

            # ---- stage-2, two stripes back: l2 accumulation ----
            fT_prev = None
            if len(pends) == 2:
                p0 = pends.pop(0)
                fT_prev = emit_l2(p0)
                pend_bsl = p0[2]

            # ---- gate: softmax over 8 experts (f32r path) ----
            pg = ps_pool.tile([8, STRIPE], f32, tag="ps")
            for k in range(KX):
                nc.tensor.matmul(
                    pg[:],
                    wb(OFF_GATE + k * NE, NE),
                    xT[k][:],
                    start=(k == 0),
                    stop=(k == KX - 1),
                )
            eT = gw_pool.tile([8, STRIPE], f32r, tag="eT")
            nc.scalar.activation(
                eT[:], pg[:], AF.Exp, bias=bslice(OFF_GATEB, parts=8), scale=1.0
            )
            psum_s = ps_pool.tile([8, STRIPE], f32, tag="ps")
            nc.tensor.matmul(psum_s[:], wr(OFF_ONES, NE, parts=8), eT[:],
                             start=True, stop=True)
            rT = gw_pool.tile([8, STRIPE], f32, tag="rT")
            nc.vector.reciprocal_approx_fast(rT[:], psum_s[:])
            gwT = gw_pool.tile([8, STRIPE], bf16, tag="gwT")
            nc.vector.tensor_mul(gwT[:], eT[:], rT[:])

            # gather gate rows onto partition 0; broadcast on idle GPSIMD
            grow = grow_pool.tile([1, NE, STRIPE], bf16, tag="grow")
            nc.scalar.dma_start(grow[:], gwT[:])

            # ---- experts: h = relu(W1.T x + b1); sh = h * gw[e] ----
            sh = []
            for e in range(NE):
                ph = ph_pool.tile([128, STRIPE], f32, tag="ph")
                for k in range(KX):
                    nc.tensor.matmul(
                        ph[:],
                        wb(OFF_W1 + e * KX * 128 + k * 128, 128),
                        xT[k][:],
                        start=(k == 0),
                        stop=(k == KX - 1),
                    )
                h = h_pool.tile([128, STRIPE], bf16, tag="h")
                nc.scalar.activation(
                    h[:], ph[:], AF.Relu, bias=bslice(OFF_B1 + e), scale=1.0
                )
                gb = gb_pool.tile([128, STRIPE], bf16, tag="gb")
                nc.gpsimd.partition_broadcast(gb[:], grow[0:1, e, :], channels=128)
                sht = sh_pool.tile([128, STRIPE], bf16, tag="sh")
                nc.vector.tensor_mul(sht[:], h[:], gb[:])
                sh.append(sht)

            if fT_prev is not None:
                emit_head(fT_prev, pend_bsl)
            pends.append((sh, gwT, bsl))

        flush = [(emit_l2(p0), p0[2]) for p0 in pends]
        for fT, bsl_ in flush:
            emit_head(fT, bsl_)

    nc.compile()
    return nc


_PROGRAM = None


def _get_program():
    global _PROGRAM
    if _PROGRAM is None:
        _PROGRAM = build_program()
    return _PROGRAM


def make_in_maps(inputs):
    """Host-side shard + layout prep: list of 8 per-core input maps."""
    wb16, wr, wbias = pack_weights(inputs)
    feats = [
        np.asarray(inputs["feat_text"], np.float32),
        np.asarray(inputs["feat_audio"], np.float32),
        np.asarray(inputs["feat_video"], np.float32),
    ]
    in_maps = []
    for c in range(NCORES):
        sl = slice(c * BL, (c + 1) * BL)
        featT = np.stack([np.ascontiguousarray(f[sl].T) for f in feats])
        in_maps.append({
            "featT": featT.astype(BF16),
            "wmatb": wb16,
            "wmatr": wr,
            "wbias": wbias,
        })
    return in_maps


def run_on_hw(inputs, trace=False):
    from concourse.bass_utils import run_bass_kernel_spmd

    nc = _get_program()
    in_maps = make_in_maps(inputs)
    res = run_bass_kernel_spmd(
        nc, in_maps, core_ids=list(range(NCORES)), trace=trace
    )
    out = np.concatenate([r["outT"].T for r in res.results], axis=0)
    return out, res


def kernel(**inputs):
    out, _ = run_on_hw(inputs, trace=False)
    return out


# revision 56
# speedup vs baseline: 1.0499x; 1.0083x over previous
"""Trainium2 Bass kernel for nn_MoEFusion (multi-modal MoE fusion MLP).

Data-parallel across 8 NeuronCores: batch dim (32768) sharded into 8
slices of 4096, all weights (<1 MB) replicated. No collectives.

On-device dataflow (per core, feature-major "T" layout everywhere):
  featT [768, 4096] (host-pre-transposed, bf16) --DMA--> SBUF
  projT[m] = proj_w[m].T @ featT[m]            (PE bf16, accum K=768)
  xT = concat_m(projT + proj_b)                (ACT bias-add PSUM->SBUF, bf16)
  gateT = exp(gate_w.T @ xT + gate_b)          (PE + ACT Exp, f32r out)
  colsum via all-ones K=8 matmul (PE), reciprocal_approx_fast (DVE)
  gwT = gateT * rsum                           (DVE, f32r)
  hT[e] = relu(W1[e].T @ xT + b1[e])           (PE bf16 + ACT Relu)
  gw bcast to 128 partitions via one-hot K=8 matmul (PE, f32r)
  shT[e] = hT[e] * gw_bcast[e]                 (DVE, bf16 out)
  fusedT = exp_b2.T @ gwT (f32r) + sum_e W2[e].T @ shT[e] (bf16), one PSUM
  penT = relu(pre_w.T @ fusedT + pre_b)        (PE + ACT)
  outT = head_w.T @ penT + head_b              (PE + ACT) --DMA--> [2, 4096]
Host re-transposes/concats to [32768, 2] fp32.

Software pipeline: the l2 accumulation and pre/head of stripe s-1 are
emitted during stripe s so the PE stream never waits on the
gate-softmax chain. bf16 moving+stationary operands stream the PE at
1 col/cycle (f32r measured ~2 cyc/row on HW); the softmax/gating path
stays f32r end-to-end so gate weights keep ~fp32 precision.
"""

import sys

if "/opt/trn_rl_repo" not in sys.path:
    sys.path.insert(0, "/opt/trn_rl_repo")

from contextlib import ExitStack

import ml_dtypes
import numpy as np

# ---- problem constants (hardcoded per contract) ----
B = 32768
NCORES = 8
BL = B // NCORES  # 4096 per core
STRIPE = 512
NM = 3
NE = 8
D_IN = 768
KIN = D_IN // 128  # 6
D_P = 128
D_X = 384
KX = D_X // 128  # 3

BF16 = ml_dtypes.bfloat16

# ---- bf16 packed weight layout (columns of [128, WBFCOLS]) ----
OFF_PROJ = 0                           # [p, m*768 + k*128 + o] = proj_w[m, k*128+p, o]
OFF_W1 = OFF_PROJ + NM * KIN * 128     # 2304
OFF_W2 = OFF_W1 + NE * KX * 128        # 5376
OFF_GATE = OFF_W2 + NE * 128           # 6400
OFF_PRE = OFF_GATE + KX * NE           # 6424
OFF_HEAD = OFF_PRE + 64                # 6488
OFF_B2B = OFF_HEAD + 2                 # 6490: [p<8, o] = exp_b2[p, o]
OFF_OHB = OFF_B2B + 128                # 6618: [p<8, e*128+q] = (p==e)
WBFCOLS = OFF_OHB + NE * 128           # 7642

# ---- f32r packed weights (softmax sum, columns of [128, WRCOLS]) ----
OFF_ONES = 0                           # [p<8, 0:8] = 1.0
WRCOLS = OFF_ONES + NE                 # 8

# ---- f32 biases (columns of [128, WBCOLS]) ----
OFF_PROJB = 0
OFF_B1 = OFF_PROJB + NM
OFF_GATEB = OFF_B1 + NE
OFF_PREB = OFF_GATEB + 1
OFF_HEADB = OFF_PREB + 1
WBCOLS = OFF_HEADB + 1                 # 14


def pack_weights(inp):
    wb16 = np.zeros((128, WBFCOLS), np.float32)
    pw = np.asarray(inp["proj_w"], np.float32)
    wb16[:, OFF_PROJ:OFF_W1] = (
        pw.reshape(NM, KIN, 128, 128).transpose(2, 0, 1, 3).reshape(128, -1)
    )
    w1 = np.asarray(inp["exp_w1"], np.float32)
    wb16[:, OFF_W1:OFF_W2] = (
        w1.reshape(NE, KX, 128, 128).transpose(2, 0, 1, 3).reshape(128, -1)
    )
    w2 = np.asarray(inp["exp_w2"], np.float32)
    wb16[:, OFF_W2:OFF_GATE] = w2.transpose(1, 0, 2).reshape(128, -1)
    gw = np.asarray(inp["gate_w"], np.float32)
    wb16[:, OFF_GATE:OFF_PRE] = (
        gw.reshape(KX, 128, NE).transpose(1, 0, 2).reshape(128, -1)
    )
    wb16[:, OFF_PRE:OFF_HEAD] = np.asarray(inp["pre_w"], np.float32)
    wb16[:64, OFF_HEAD:OFF_B2B] = np.asarray(inp["head_w"], np.float32)
    wb16[:8, OFF_B2B:OFF_OHB] = np.asarray(inp["exp_b2"], np.float32)
    oh = np.zeros((8, NE, 128), np.float32)
    for e in range(NE):
        oh[e, e, :] = 1.0
    wb16[:8, OFF_OHB:WBFCOLS] = oh.reshape(8, -1)
    wb16 = wb16.astype(BF16)

    wr = np.zeros((128, WRCOLS), np.float32)
    wr[:8, OFF_ONES:OFF_ONES + NE] = 1.0

    wbias = np.zeros((128, WBCOLS), np.float32)
    wbias[:, OFF_PROJB:OFF_B1] = np.asarray(inp["proj_b"], np.float32).T
    wbias[:, OFF_B1:OFF_GATEB] = np.asarray(inp["exp_b1"], np.float32).T
    wbias[:8, OFF_GATEB] = np.asarray(inp["gate_b"], np.float32)
    wbias[:64, OFF_PREB] = np.asarray(inp["pre_b"], np.float32)
    wbias[:2, OFF_HEADB] = np.asarray(inp["head_b"], np.float32)
    return wb16, wr, wbias


def build_program(n_stripes=BL // STRIPE):
    """Build the per-core Bass program (identical on all cores)."""
    import concourse.bacc as bacc
    import concourse.mybir as mybir
    import concourse.tile as tile

    f32 = mybir.dt.float32
    f32r = mybir.dt.float32r
    bf16 = mybir.dt.bfloat16
    AF = mybir.ActivationFunctionType
    bl = n_stripes * STRIPE

    nc = bacc.Bacc(
        "TRN2",
        target_bir_lowering=False,
        debug=False,
        enable_asserts=False,
    )

    featT = nc.dram_tensor("featT", [NM, D_IN, bl], bf16, kind="ExternalInput").ap()
    wmatb = nc.dram_tensor("wmatb", [128, WBFCOLS], bf16, kind="ExternalInput").ap()
    wmatr = nc.dram_tensor("wmatr", [128, WRCOLS], f32r, kind="ExternalInput").ap()
    wbias = nc.dram_tensor("wbias", [128, WBCOLS], f32, kind="ExternalInput").ap()
    outT = nc.dram_tensor("outT", [2, bl], f32, kind="ExternalOutput").ap()

    with tile.TileContext(nc) as tc, ExitStack() as ctx:
        wp_pool = ctx.enter_context(tc.tile_pool(name="wp", bufs=1))
        feat_pool = ctx.enter_context(tc.tile_pool(name="feat", bufs=18))
        x_pool = ctx.enter_context(tc.tile_pool(name="x", bufs=6))
        gw_pool = ctx.enter_context(tc.tile_pool(name="gw", bufs=4))
        grow_pool = ctx.enter_context(tc.tile_pool(name="grow", bufs=4))
        gb_pool = ctx.enter_context(tc.tile_pool(name="gb", bufs=6))
        h_pool = ctx.enter_context(tc.tile_pool(name="h", bufs=10))
        sh_pool = ctx.enter_context(tc.tile_pool(name="sh", bufs=26))
        f_pool = ctx.enter_context(tc.tile_pool(name="f", bufs=2))
        pen_pool = ctx.enter_context(tc.tile_pool(name="pen", bufs=2))
        o_pool = ctx.enter_context(tc.tile_pool(name="o", bufs=2))

        px_pool = ctx.enter_context(tc.tile_pool(name="px", bufs=2, space="PSUM"))
        ph_pool = ctx.enter_context(tc.tile_pool(name="ph", bufs=3, space="PSUM"))
        pf_pool = ctx.enter_context(tc.tile_pool(name="pf", bufs=2, space="PSUM"))
        ps_pool = ctx.enter_context(tc.tile_pool(name="ps", bufs=1, space="PSUM"))

        # preload packed weights once (proj block first so MMs start early)
        Wb = wp_pool.tile([128, WBFCOLS], bf16)
        nc.scalar.dma_start(Wb[:, :OFF_W1], wmatb[:, :OFF_W1])
        nc.scalar.dma_start(Wb[:, OFF_W1:], wmatb[:, OFF_W1:])
        Wr = wp_pool.tile([128, WRCOLS], f32r)
        nc.scalar.dma_start(Wr[:], wmatr[:])
        Bz = wp_pool.tile([128, WBCOLS], f32)
        nc.scalar.dma_start(Bz[:], wbias[:])


        def wb(off, n, parts=128):
            return Wb[:parts, off : off + n]

        def wr(off, n, parts=128):
            return Wr[:parts, off : off + n]

        def bslice(off, parts=128):
            return Bz[:parts, off : off + 1]

        featT_t = featT.rearrange("m (k p) b -> m p k b", p=128)

        pends = []  # (sh, gwT, bsl) of the previous two stripes

        def emit_l2(pend):
            sh, gwT, bsl = pend
            pf = pf_pool.tile([128, STRIPE], f32, tag="pf")
            nc.tensor.matmul(
                pf[:], wb(OFF_B2B, 128, parts=8), gwT[:],
                start=True, stop=False,
            )
            for e in range(NE):
                nc.tensor.matmul(
                    pf[:],
                    wb(OFF_W2 + e * 128, 128),
                    sh[e][:],
                    start=False,
                    stop=(e == NE - 1),
                )
            fT = f_pool.tile([128, STRIPE], bf16, tag="f")
            nc.scalar.copy(fT[:], pf[:])
            return fT

        def emit_head(fT, bsl):
            pp = ps_pool.tile([64, STRIPE], f32, tag="ps")
            nc.tensor.matmul(pp[:], wb(OFF_PRE, 64), fT[:],
                             start=True, stop=True)
            pen = pen_pool.tile([64, STRIPE], bf16, tag="pen")
            nc.scalar.activation(
                pen[:], pp[:], AF.Relu, bias=bslice(OFF_PREB, parts=64), scale=1.0
            )
            po = ps_pool.tile([2, STRIPE], f32, tag="ps")
            nc.tensor.matmul(po[:], wb(OFF_HEAD, 2, parts=64), pen[:],
                             start=True, stop=True)
            ot = o_pool.tile([2, STRIPE], f32, tag="o")
            nc.scalar.activation(
                ot[:], po[:], AF.Identity, bias=bslice(OFF_HEADB, parts=2),
                scale=1.0,
            )
            nc.scalar.dma_start(outT[:, bsl], ot[:])

        for s in range(n_stripes):
            bsl = slice(s * STRIPE, (s + 1) * STRIPE)

            # ---- load features (0.75 MB per modality) ----
            ft = []
            h2 = KIN // 2
            for m in range(NM):
                ta = feat_pool.tile([128, h2, STRIPE], bf16, tag="feat")
                nc.sync.dma_start(ta[:], featT_t[m, :, :h2, bsl])
                tb = feat_pool.tile([128, KIN - h2, STRIPE], bf16, tag="feat")
                nc.sync.dma_start(tb[:], featT_t[m, :, h2:, bsl])
                ft.append((ta, tb))

            # ---- per-modality projection -> xT chunks ----
            xT = []
            for m in range(NM):
                px = px_pool.tile([128, STRIPE], f32, tag="px")
                for k in range(KIN):
                    src_t = ft[m][0] if k < h2 else ft[m][1]
                    nc.tensor.matmul(
                        px[:],
                        wb(OFF_PROJ + m * KIN * 128 + k * 128, 128),
                        src_t[:, k % h2, :],
                        start=(k == 0),
                        stop=(k == KIN - 1),
                    )
                xt = x_pool.tile([128, STRIPE], bf16, tag="x")
                nc.scalar.activation(
                    xt[:], px[:], AF.Identity,
                    bias=bslice(OFF_PROJB + m), scale=1.0,
                )
                xT.append(xt)

            # ---- stage-2, three stripes back: l2 accumulation ----
            fT_prev = None
            if len(pends) == 3:
                p0 = pends.pop(0)
                fT_prev = emit_l2(p0)
                pend_bsl = p0[2]

            # ---- gate: softmax over 8 experts (f32r path) ----
            pg = ps_pool.tile([8, STRIPE], f32, tag="ps")
            for k in range(KX):
                nc.tensor.matmul(
                    pg[:],
                    wb(OFF_GATE + k * NE, NE),
                    xT[k][:],
                    start=(k == 0),
                    stop=(k == KX - 1),
                )
            eT = gw_pool.tile([8, STRIPE], f32r, tag="eT")
            nc.scalar.activation(
                eT[:], pg[:], AF.Exp, bias=bslice(OFF_GATEB, parts=8), scale=1.0
            )
            psum_s = ps_pool.tile([8, STRIPE], f32, tag="ps")
            nc.tensor.matmul(psum_s[:], wr(OFF_ONES, NE, parts=8), eT[:],
                             start=True, stop=True)
            rT = gw_pool.tile([8, STRIPE], f32, tag="rT")
            nc.vector.reciprocal_approx_fast(rT[:], psum_s[:])
            gwT = gw_pool.tile([8, STRIPE], bf16, tag="gwT")
            nc.vector.tensor_mul(gwT[:], eT[:], rT[:])

            # gather gate rows onto partition 0; broadcast on idle GPSIMD
            grow = grow_pool.tile([1, NE, STRIPE], bf16, tag="grow")
            nc.scalar.dma_start(grow[:], gwT[:])

            # ---- experts: h = relu(W1.T x + b1); sh = h * gw[e] ----
            sh = []
            for e in range(NE):
                ph = ph_pool.tile([128, STRIPE], f32, tag="ph")
                for k in range(KX):
                    nc.tensor.matmul(
                        ph[:],
                        wb(OFF_W1 + e * KX * 128 + k * 128, 128),
                        xT[k][:],
                        start=(k == 0),
                        stop=(k == KX - 1),
                    )
                h = h_pool.tile([128, STRIPE], bf16, tag="h")
                nc.scalar.activation(
                    h[:], ph[:], AF.Relu, bias=bslice(OFF_B1 + e), scale=1.0
                )
                gb = gb_pool.tile([128, STRIPE], bf16, tag="gb")
                nc.gpsimd.partition_broadcast(gb[:], grow[0:1, e, :], channels=128)
                sht = sh_pool.tile([128, STRIPE], bf16, tag="sh")
                nc.vector.tensor_mul(sht[:], h[:], gb[:])
                sh.append(sht)

            if fT_prev is not None:
                emit_head(fT_prev, pend_bsl)
            pends.append((sh, gwT, bsl))

        flush = [(emit_l2(p0), p0[2]) for p0 in pends]
        for fT, bsl_ in flush:
            emit_head(fT, bsl_)

    nc.compile()
    return nc


_PROGRAM = None


def _get_program():
    global _PROGRAM
    if _PROGRAM is None:
        _PROGRAM = build_program()
    return _PROGRAM


def make_in_maps(inputs):
    """Host-side shard + layout prep: list of 8 per-core input maps."""
    wb16, wr, wbias = pack_weights(inputs)
    feats = [
        np.asarray(inputs["feat_text"], np.float32),
        np.asarray(inputs["feat_audio"], np.float32),
        np.asarray(inputs["feat_video"], np.float32),
    ]
    in_maps = []
    for c in range(NCORES):
        sl = slice(c * BL, (c + 1) * BL)
        featT = np.stack([np.ascontiguousarray(f[sl].T) for f in feats])
        in_maps.append({
            "featT": featT.astype(BF16),
            "wmatb": wb16,
            "wmatr": wr,
            "wbias": wbias,
        })
    return in_maps


def run_on_hw(inputs, trace=False):
    from concourse.bass_utils import run_bass_kernel_spmd

    nc = _get_program()
    in_maps = make_in_maps(inputs)
    res = run_bass_kernel_spmd(
        nc, in_maps, core_ids=list(range(NCORES)), trace=trace
    )
    out = np.concatenate([r["outT"].T for r in res.results], axis=0)
    return out, res


def kernel(**inputs):
    out, _ = run_on_hw(inputs, trace=False)
    return out


# revision 57
# speedup vs baseline: 1.0610x; 1.0106x over previous
"""Trainium2 Bass kernel for nn_MoEFusion (multi-modal MoE fusion MLP).

Data-parallel across 8 NeuronCores: batch dim (32768) sharded into 8
slices of 4096, all weights (<1 MB) replicated. No collectives.

On-device dataflow (per core, feature-major "T" layout everywhere):
  featT [768, 4096] (host-pre-transposed, bf16) --DMA--> SBUF
  projT[m] = proj_w[m].T @ featT[m]            (PE bf16, accum K=768)
  xT = concat_m(projT + proj_b)                (ACT bias-add PSUM->SBUF, bf16)
  gateT = exp(gate_w.T @ xT + gate_b)          (PE + ACT Exp, f32r out)
  colsum via all-ones K=8 matmul (PE), reciprocal_approx_fast (DVE)
  gwT = gateT * rsum                           (DVE, f32r)
  hT[e] = relu(W1[e].T @ xT + b1[e])           (PE bf16 + ACT Relu)
  gw bcast to 128 partitions via one-hot K=8 matmul (PE, f32r)
  shT[e] = hT[e] * gw_bcast[e]                 (DVE, bf16 out)
  fusedT = exp_b2.T @ gwT (f32r) + sum_e W2[e].T @ shT[e] (bf16), one PSUM
  penT = relu(pre_w.T @ fusedT + pre_b)        (PE + ACT)
  outT = head_w.T @ penT + head_b              (PE + ACT) --DMA--> [2, 4096]
Host re-transposes/concats to [32768, 2] fp32.

Software pipeline: the l2 accumulation and pre/head of stripe s-1 are
emitted during stripe s so the PE stream never waits on the
gate-softmax chain. bf16 moving+stationary operands stream the PE at
1 col/cycle (f32r measured ~2 cyc/row on HW); the softmax/gating path
stays f32r end-to-end so gate weights keep ~fp32 precision.
"""

import sys

if "/opt/trn_rl_repo" not in sys.path:
    sys.path.insert(0, "/opt/trn_rl_repo")

from contextlib import ExitStack

import ml_dtypes
import numpy as np

# ---- problem constants (hardcoded per contract) ----
B = 32768
NCORES = 8
BL = B // NCORES  # 4096 per core
STRIPE = 512
NM = 3
NE = 8
D_IN = 768
KIN = D_IN // 128  # 6
D_P = 128
D_X = 384
KX = D_X // 128  # 3

BF16 = ml_dtypes.bfloat16

# ---- bf16 packed weight layout (columns of [128, WBFCOLS]) ----
OFF_PROJ = 0                           # [p, m*768 + k*128 + o] = proj_w[m, k*128+p, o]
OFF_W1 = OFF_PROJ + NM * KIN * 128     # 2304
OFF_W2 = OFF_W1 + NE * KX * 128        # 5376
OFF_GATE = OFF_W2 + NE * 128           # 6400
OFF_PRE = OFF_GATE + KX * NE           # 6424
OFF_HEAD = OFF_PRE + 64                # 6488
OFF_B2B = OFF_HEAD + 2                 # 6490: [p<8, o] = exp_b2[p, o]
OFF_OHB = OFF_B2B + 128                # 6618: [p<8, e*128+q] = (p==e)
WBFCOLS = OFF_OHB + NE * 128           # 7642

# ---- f32r packed weights (softmax sum, columns of [128, WRCOLS]) ----
OFF_ONES = 0                           # [p<8, 0:8] = 1.0
WRCOLS = OFF_ONES + NE                 # 8

# ---- f32 biases (columns of [128, WBCOLS]) ----
OFF_PROJB = 0
OFF_B1 = OFF_PROJB + NM
OFF_GATEB = OFF_B1 + NE
OFF_PREB = OFF_GATEB + 1
OFF_HEADB = OFF_PREB + 1
WBCOLS = OFF_HEADB + 1                 # 14


def pack_weights(inp):
    wb16 = np.zeros((128, WBFCOLS), np.float32)
    pw = np.asarray(inp["proj_w"], np.float32)
    wb16[:, OFF_PROJ:OFF_W1] = (
        pw.reshape(NM, KIN, 128, 128).transpose(2, 0, 1, 3).reshape(128, -1)
    )
    w1 = np.asarray(inp["exp_w1"], np.float32)
    wb16[:, OFF_W1:OFF_W2] = (
        w1.reshape(NE, KX, 128, 128).transpose(2, 0, 1, 3).reshape(128, -1)
    )
    w2 = np.asarray(inp["exp_w2"], np.float32)
    wb16[:, OFF_W2:OFF_GATE] = w2.transpose(1, 0, 2).reshape(128, -1)
    gw = np.asarray(inp["gate_w"], np.float32)
    wb16[:, OFF_GATE:OFF_PRE] = (
        gw.reshape(KX, 128, NE).transpose(1, 0, 2).reshape(128, -1)
    )
    wb16[:, OFF_PRE:OFF_HEAD] = np.asarray(inp["pre_w"], np.float32)
    wb16[:64, OFF_HEAD:OFF_B2B] = np.asarray(inp["head_w"], np.float32)
    wb16[:8, OFF_B2B:OFF_OHB] = np.asarray(inp["exp_b2"], np.float32)
    oh = np.zeros((8, NE, 128), np.float32)
    for e in range(NE):
        oh[e, e, :] = 1.0
    wb16[:8, OFF_OHB:WBFCOLS] = oh.reshape(8, -1)
    wb16 = wb16.astype(BF16)

    wr = np.zeros((128, WRCOLS), np.float32)
    wr[:8, OFF_ONES:OFF_ONES + NE] = 1.0

    wbias = np.zeros((128, WBCOLS), np.float32)
    wbias[:, OFF_PROJB:OFF_B1] = np.asarray(inp["proj_b"], np.float32).T
    wbias[:, OFF_B1:OFF_GATEB] = np.asarray(inp["exp_b1"], np.float32).T
    wbias[:8, OFF_GATEB] = np.asarray(inp["gate_b"], np.float32)
    wbias[:64, OFF_PREB] = np.asarray(inp["pre_b"], np.float32)
    wbias[:2, OFF_HEADB] = np.asarray(inp["head_b"], np.float32)
    return wb16, wr, wbias


def build_program(n_stripes=BL // STRIPE):
    """Build the per-core Bass program (identical on all cores)."""
    import concourse.bacc as bacc
    import concourse.mybir as mybir
    import concourse.tile as tile

    f32 = mybir.dt.float32
    f32r = mybir.dt.float32r
    bf16 = mybir.dt.bfloat16
    AF = mybir.ActivationFunctionType
    bl = n_stripes * STRIPE

    nc = bacc.Bacc(
        "TRN2",
        target_bir_lowering=False,
        debug=False,
        enable_asserts=False,
    )

    featT = nc.dram_tensor("featT", [NM, D_IN, bl], bf16, kind="ExternalInput").ap()
    wmatb = nc.dram_tensor("wmatb", [128, WBFCOLS], bf16, kind="ExternalInput").ap()
    wmatr = nc.dram_tensor("wmatr", [128, WRCOLS], f32r, kind="ExternalInput").ap()
    wbias = nc.dram_tensor("wbias", [128, WBCOLS], f32, kind="ExternalInput").ap()
    outT = nc.dram_tensor("outT", [2, bl], f32, kind="ExternalOutput").ap()

    with tile.TileContext(nc) as tc, ExitStack() as ctx:
        wp_pool = ctx.enter_context(tc.tile_pool(name="wp", bufs=1))
        feat_pool = ctx.enter_context(tc.tile_pool(name="feat", bufs=18))
        x_pool = ctx.enter_context(tc.tile_pool(name="x", bufs=6))
        gw_pool = ctx.enter_context(tc.tile_pool(name="gw", bufs=4))
        grow_pool = ctx.enter_context(tc.tile_pool(name="grow", bufs=4))
        gb_pool = ctx.enter_context(tc.tile_pool(name="gb", bufs=6))
        h_pool = ctx.enter_context(tc.tile_pool(name="h", bufs=10))
        sh_pool = ctx.enter_context(tc.tile_pool(name="sh", bufs=26))
        f_pool = ctx.enter_context(tc.tile_pool(name="f", bufs=2))
        pen_pool = ctx.enter_context(tc.tile_pool(name="pen", bufs=2))
        o_pool = ctx.enter_context(tc.tile_pool(name="o", bufs=2))

        px_pool = ctx.enter_context(tc.tile_pool(name="px", bufs=2, space="PSUM"))
        ph_pool = ctx.enter_context(tc.tile_pool(name="ph", bufs=3, space="PSUM"))
        pf_pool = ctx.enter_context(tc.tile_pool(name="pf", bufs=2, space="PSUM"))
        ps_pool = ctx.enter_context(tc.tile_pool(name="ps", bufs=1, space="PSUM"))

        # preload packed weights once. The two tiny tensors go first on
        # the sync ring to absorb the queue's cold first-transfer penalty
        # before the feature streams start; proj weights lead the scalar
        # ring so matmuls can start early.
        Bz = wp_pool.tile([128, WBCOLS], f32)
        nc.sync.dma_start(Bz[:], wbias[:])
        Wr = wp_pool.tile([128, WRCOLS], f32r)
        nc.sync.dma_start(Wr[:], wmatr[:])
        Wb = wp_pool.tile([128, WBFCOLS], bf16)
        nc.scalar.dma_start(Wb[:, :OFF_W1], wmatb[:, :OFF_W1])
        nc.scalar.dma_start(Wb[:, OFF_W1:], wmatb[:, OFF_W1:])


        def wb(off, n, parts=128):
            return Wb[:parts, off : off + n]

        def wr(off, n, parts=128):
            return Wr[:parts, off : off + n]

        def bslice(off, parts=128):
            return Bz[:parts, off : off + 1]

        featT_t = featT.rearrange("m (k p) b -> m p k b", p=128)

        pends = []  # (sh, gwT, bsl) of the previous two stripes

        def emit_l2(pend):
            sh, gwT, bsl = pend
            pf = pf_pool.tile([128, STRIPE], f32, tag="pf")
            nc.tensor.matmul(
                pf[:], wb(OFF_B2B, 128, parts=8), gwT[:],
                start=True, stop=False,
            )
            for e in range(NE):
                nc.tensor.matmul(
                    pf[:],
                    wb(OFF_W2 + e * 128, 128),
                    sh[e][:],
                    start=False,
                    stop=(e == NE - 1),
                )
            fT = f_pool.tile([128, STRIPE], bf16, tag="f")
            nc.scalar.copy(fT[:], pf[:])
            return fT

        def emit_head(fT, bsl):
            pp = ps_pool.tile([64, STRIPE], f32, tag="ps")
            nc.tensor.matmul(pp[:], wb(OFF_PRE, 64), fT[:],
                             start=True, stop=True)
            pen = pen_pool.tile([64, STRIPE], bf16, tag="pen")
            nc.scalar.activation(
                pen[:], pp[:], AF.Relu, bias=bslice(OFF_PREB, parts=64), scale=1.0
            )
            po = ps_pool.tile([2, STRIPE], f32, tag="ps")
            nc.tensor.matmul(po[:], wb(OFF_HEAD, 2, parts=64), pen[:],
                             start=True, stop=True)
            ot = o_pool.tile([2, STRIPE], f32, tag="o")
            nc.scalar.activation(
                ot[:], po[:], AF.Identity, bias=bslice(OFF_HEADB, parts=2),
                scale=1.0,
            )
            nc.scalar.dma_start(outT[:, bsl], ot[:])

        for s in range(n_stripes):
            bsl = slice(s * STRIPE, (s + 1) * STRIPE)

            # ---- load features (0.75 MB per modality) ----
            ft = []
            h2 = KIN // 2
            for m in range(NM):
                ta = feat_pool.tile([128, h2, STRIPE], bf16, tag="feat")
                nc.sync.dma_start(ta[:], featT_t[m, :, :h2, bsl])
                tb = feat_pool.tile([128, KIN - h2, STRIPE], bf16, tag="feat")
                nc.sync.dma_start(tb[:], featT_t[m, :, h2:, bsl])
                ft.append((ta, tb))

            # ---- per-modality projection -> xT chunks ----
            xT = []
            for m in range(NM):
                px = px_pool.tile([128, STRIPE], f32, tag="px")
                for k in range(KIN):
                    src_t = ft[m][0] if k < h2 else ft[m][1]
                    nc.tensor.matmul(
                        px[:],
                        wb(OFF_PROJ + m * KIN * 128 + k * 128, 128),
                        src_t[:, k % h2, :],
                        start=(k == 0),
                        stop=(k == KIN - 1),
                    )
                xt = x_pool.tile([128, STRIPE], bf16, tag="x")
                nc.scalar.activation(
                    xt[:], px[:], AF.Identity,
                    bias=bslice(OFF_PROJB + m), scale=1.0,
                )
                xT.append(xt)

            # ---- stage-2, three stripes back: l2 accumulation ----
            fT_prev = None
            if len(pends) == 3:
                p0 = pends.pop(0)
                fT_prev = emit_l2(p0)
                pend_bsl = p0[2]

            # ---- gate: softmax over 8 experts (f32r path) ----
            pg = ps_pool.tile([8, STRIPE], f32, tag="ps")
            for k in range(KX):
                nc.tensor.matmul(
                    pg[:],
                    wb(OFF_GATE + k * NE, NE),
                    xT[k][:],
                    start=(k == 0),
                    stop=(k == KX - 1),
                )
            eT = gw_pool.tile([8, STRIPE], f32r, tag="eT")
            nc.scalar.activation(
                eT[:], pg[:], AF.Exp, bias=bslice(OFF_GATEB, parts=8), scale=1.0
            )
            psum_s = ps_pool.tile([8, STRIPE], f32, tag="ps")
            nc.tensor.matmul(psum_s[:], wr(OFF_ONES, NE, parts=8), eT[:],
                             start=True, stop=True)
            rT = gw_pool.tile([8, STRIPE], f32, tag="rT")
            nc.vector.reciprocal_approx_fast(rT[:], psum_s[:])
            gwT = gw_pool.tile([8, STRIPE], bf16, tag="gwT")
            nc.vector.tensor_mul(gwT[:], eT[:], rT[:])

            # gather gate rows onto partition 0; broadcast on idle GPSIMD
            grow = grow_pool.tile([1, NE, STRIPE], bf16, tag="grow")
            nc.scalar.dma_start(grow[:], gwT[:])

            # ---- experts: h = relu(W1.T x + b1); sh = h * gw[e] ----
            sh = []
            for e in range(NE):
                ph = ph_pool.tile([128, STRIPE], f32, tag="ph")
                for k in range(KX):
                    nc.tensor.matmul(
                        ph[:],
                        wb(OFF_W1 + e * KX * 128 + k * 128, 128),
                        xT[k][:],
                        start=(k == 0),
                        stop=(k == KX - 1),
                    )
                h = h_pool.tile([128, STRIPE], bf16, tag="h")
                nc.scalar.activation(
                    h[:], ph[:], AF.Relu, bias=bslice(OFF_B1 + e), scale=1.0
                )
                gb = gb_pool.tile([128, STRIPE], bf16, tag="gb")
                nc.gpsimd.partition_broadcast(gb[:], grow[0:1, e, :], channels=128)
                sht = sh_pool.tile([128, STRIPE], bf16, tag="sh")
                nc.vector.tensor_mul(sht[:], h[:], gb[:])
                sh.append(sht)

            if fT_prev is not None:
                emit_head(fT_prev, pend_bsl)
            pends.append((sh, gwT, bsl))

        flush = [(emit_l2(p0), p0[2]) for p0 in pends]
        for fT, bsl_ in flush:
            emit_head(fT, bsl_)

    nc.compile()
    return nc


_PROGRAM = None


def _get_program():
    global _PROGRAM
    if _PROGRAM is None:
        _PROGRAM = build_program()
    return _PROGRAM


def make_in_maps(inputs):
    """Host-side shard + layout prep: list of 8 per-core input maps."""
    wb16, wr, wbias = pack_weights(inputs)
    feats = [
        np.asarray(inputs["feat_text"], np.float32),
        np.asarray(inputs["feat_audio"], np.float32),
        np.asarray(inputs["feat_video"], np.float32),
    ]
    in_maps = []
    for c in range(NCORES):
        sl = slice(c * BL, (c + 1) * BL)
        featT = np.stack([np.ascontiguousarray(f[sl].T) for f in feats])
        in_maps.append({
            "featT": featT.astype(BF16),
            "wmatb": wb16,
            "wmatr": wr,
            "wbias": wbias,
        })
    return in_maps


def run_on_hw(inputs, trace=False):
    from concourse.bass_utils import run_bass_kernel_spmd

    nc = _get_program()
    in_maps = make_in_maps(inputs)
    res = run_bass_kernel_spmd(
        nc, in_maps, core_ids=list(range(NCORES)), trace=trace
    )
    out = np.concatenate([r["outT"].T for r in res.results], axis=0)
    return out, res


def kernel(**inputs):
    out, _ = run_on_hw(inputs, trace=False)
    return out
